# revision 1
# baseline (speedup 1.0000x reference)
"""HD95 loss kernel for Trainium2 (Bass/Tile), 8 NeuronCores.

Reference semantics: per image, threshold pred/true at 0.5, compact nonzero
pixel indices in row-major order, split each point list into blocks of 1000,
and for every (point, opposite-side block) pair take the min Euclidean
distance; the HD95 is the 95th linear-interpolation quantile over all finite
such mins (both directions), averaged over the batch.

Device algorithm (per image & direction, "queries" vs "ref blocks"):
separable squared-EDT. All coordinates are integers < 96, and every operand
is decomposed into bf16-exact integer parts (squares split into a multiple
of 128 plus a <128 remainder), so every matmul product is exact in the fp32
PSUM accumulator and the result is bit-exact vs the reference.

  stage 1:  g[x, c] = min_{a : pixel(b0+c, a) in block} (x-a)^2
            contraction-5 bf16 matmul ([x2h,x2l,x,1,1] x [1,1,-2a,a2h,a2l],
            sentinel columns [0,0,0,2^26,0]) over a <=24-row candidate
            window per block, then a DVE min-reduce per 96-col group.
  split:    g -> g_hi (multiple of 128) + g_lo (<128), both bf16-exact.
  stage 2:  min d^2(q, blk) = min_c ( (y_q - (b0+c))^2 + g[x_q, c] )
            three accumulating bf16 matmuls per 128-query tile:
            onehot(x_q) @ g_hi, onehot(x_q) @ g_lo (the g gather), and
            [y2h,y2l,y,1,1] @ [1,1,-2b,b2h,b2l] (the (y-b)^2 term);
            then a DVE min-reduce over the 24 candidates of each block.

Core mapping: 8 cores = 4 (image x direction) jobs x 2 halves of 2560
query slots. Host does the O(N) compaction/feature build and the final
O(50k) quantile; device does all O(K x window) distance work.
"""

import numpy as np

H = 96
W = 96
BLK = 1000        # reference cdist block size
NBLK = 5          # blocks per side (asserted from the data regime)
CAND = 24         # candidate image rows per block window (spans <= 23 here)
CHUNK = 384       # stage-1 matmul free size (4 candidate rows)
CPG = 3           # stage-1 matmul chunks per group (psum tile = 3 banks)
NG1 = NBLK * CAND * 96 // (CHUNK * CPG)  # 10 stage-1 groups, 12 cands each
QHALF = 2560      # query slots per core (20 tiles of 128)
NTILES = QHALF // 128
BIG = float(2 ** 26)  # sentinel (bf16-exact, >> max real d^2 of 18050)
NCORES = 8

_CACHE = {}


def _build_nc():
    import concourse.bacc as bacc
    import concourse.mybir as mybir
    import concourse.tile as tile

    f32 = mybir.dt.float32
    bf16 = mybir.dt.bfloat16
    # Bacc (not raw Bass): its compile() runs move_matmul_waits_to_ldweights
    # + generate_event_semaphores, which legalize multi-wait instructions
    # (TRN2 allows at most one sync wait per instruction).
    nc = bacc.Bacc("TRN2", target_bir_lowering=False, debug=False)

    s1_pack = nc.declare_dram_parameter(
        "s1_pack", [5, 96 + NBLK * CAND * 96], bf16, isOutput=False
    )
    s2_lhsT = nc.declare_dram_parameter(
        "s2_lhsT", [101, NTILES * 128], bf16, isOutput=False
    )
    s2_rtop = nc.declare_dram_parameter(
        "s2_rtop", [5, NBLK * CAND], bf16, isOutput=False
    )
    mins = nc.declare_dram_parameter(
        "mins", [128, NTILES * NBLK], f32, isOutput=True
    )

    X = mybir.AxisListType.X
    MIN = mybir.AluOpType.min

    with tile.TileContext(nc) as tc:
        with (
            tc.tile_pool(name="const", bufs=1) as const,
            tc.tile_pool(name="ps1", bufs=2, space="PSUM") as ps1,
            tc.tile_pool(name="ps2", bufs=2, space="PSUM") as ps2,
        ):
            t_s1 = const.tile([5, 96 + NBLK * CAND * 96], bf16)
            t_s2_lhsT = const.tile([101, NTILES * 128], bf16)
            t_rhs2 = const.tile([96, NBLK * CAND], f32)
            t_tmp32 = const.tile([96, NBLK * CAND], f32)
            t_gh32 = const.tile([96, NBLK * CAND], f32)
            # rows 0..95: g_hi (ACT-written); rows 96..100: rtop (DMA)
            t_ghr = const.tile([101, NBLK * CAND], bf16)
            t_gl = const.tile([96, NBLK * CAND], bf16)
            t_out = const.tile([128, NTILES * NBLK], f32)
            t_s1_lhsT = t_s1[:, 0:96]

            # split the critical-path DMA across 4 HWDGE queues
            n1 = 96 + NBLK * CAND * 96
            for i in range(4):
                sl = slice(i * n1 // 4, (i + 1) * n1 // 4)
                nc.sync.dma_start(t_s1[:, sl], s1_pack[:, sl])
            nc.sync.dma_start(t_s2_lhsT[:], s2_lhsT[:])
            nc.sync.dma_start(t_ghr[96:101, :], s2_rtop[:])

            # stage 1: g[x, (blk, cand)] -> t_rhs2, 12 candidate rows/group
            for gi in range(NG1):
                ps = ps1.tile([96, CPG, 512], f32, tag="ps1")
                for k in range(CPG):
                    c0 = 96 + (gi * CPG + k) * CHUNK
                    nc.tensor.matmul(
                        ps[:, k, 0:CHUNK],
                        t_s1_lhsT,
                        t_s1[:, c0 : c0 + CHUNK],
                        start=True,
                        stop=True,
                    )
                # [96, 3, 384] -> [96, 3, 4, 96], min over innermost
                red_in = ps[:, :, 0:CHUNK].rearrange("p c (u a) -> p c u a", a=96)
                o0 = gi * (CAND // 2)
                nc.vector.tensor_reduce(
                    t_rhs2[0:96, o0 : o0 + CAND // 2], red_in, axis=X, op=MIN
                )

            # split g into bf16-exact hi/lo parts for the stage-2 gather:
            # hi = round(g/128)*128 via the +2^23 float-rounding trick,
            # lo = g - hi in [-64, 64) -- both exact in bf16, sum exact.
            # The rounding runs on the idle Scalar engine (out = Copy(
            # in*scale + bias)); only the subtract needs the Vector engine.
            P23 = float(2 ** 23)
            COPY = mybir.ActivationFunctionType.Copy
            nc.scalar.activation(
                t_tmp32[:], t_rhs2[:], COPY, bias=P23, scale=1.0 / 128.0
            )
            nc.scalar.activation(
                t_gh32[:], t_tmp32[:], COPY, bias=-P23 * 128.0, scale=128.0
            )
            nc.scalar.activation(t_ghr[0:96, :], t_gh32[:], COPY)
            nc.vector.tensor_sub(t_gl[:], t_rhs2[:], t_gh32[:])

            # stage 2: two tiles per PSUM bank; per tile two accumulating
            # matmuls ([onehot;yfeat] @ [g_hi;rtop], then onehot @ g_lo),
            # then one paired min-reduce
            for t2 in range(NTILES // 2):
                ps_o = ps2.tile([128, 2, NBLK, CAND], f32, tag="ps2")
                for h in range(2):
                    t = 2 * t2 + h
                    ts = slice(t * 128, (t + 1) * 128)
                    nc.tensor.matmul(
                        ps_o[:, h, :, :], t_s2_lhsT[:, ts], t_ghr[:],
                        start=True, stop=False,
                    )
                    nc.tensor.matmul(
                        ps_o[:, h, :, :], t_s2_lhsT[0:96, ts], t_gl[:],
                        start=False, stop=True,
                    )
                nc.vector.tensor_reduce(
                    t_out[:, t2 * 2 * NBLK : (t2 + 1) * 2 * NBLK],
                    ps_o[:, :, :, :], axis=X, op=MIN,
                )

            nc.sync.dma_start(mins[:], t_out[:])

    nc.compile()
    return nc


def _get_nc():
    if "nc" not in _CACHE:
        _CACHE["nc"] = _build_nc()
    return _CACHE["nc"]


def _bf16(a):
    from ml_dtypes import bfloat16

    return np.asarray(a, np.float32).astype(bfloat16)


def _hilo(v):
    """Split integer-valued array into (multiple-of-128, remainder<128)."""
    v = np.asarray(v, np.float64)
    lo = np.mod(v, 128.0)
    return (v - lo).astype(np.float32), lo.astype(np.float32)


def _side_points(img):
    """Compacted nonzero pixel coords, row-major ascending (matches
    jnp.nonzero order)."""
    m = (np.asarray(img) > 0.5).reshape(-1)
    idx = np.nonzero(m)[0]
    ys = (idx // W).astype(np.int64)
    xs = (idx % W).astype(np.int64)
    return ys, xs


def _feat5_queries(vals):
    """[v2h, v2l, v, 1, 1] feature rows for the squared-term side."""
    v = np.asarray(vals, np.float64)
    h, l = _hilo(v * v)
    one = np.ones_like(v, np.float32)
    return np.stack([h, l, v.astype(np.float32), one, one])


def _feat5_refs(vals):
    """[1, 1, -2v, v2h, v2l] feature rows for the reference side."""
    v = np.asarray(vals, np.float64)
    h, l = _hilo(v * v)
    one = np.ones_like(v, np.float32)
    return np.stack([one, one, (-2.0 * v).astype(np.float32), h, l])


def _build_core_inputs(q_ys, q_xs, r_ys, r_xs):
    """Host-side feature build for one (image, direction) job.

    q_*: query points (cnt_q), r_*: reference points (cnt_r, split into
    NBLK blocks of BLK in compacted order). Returns two per-core input
    maps, or None if the data falls outside the compiled regime.
    """
    cnt_q, cnt_r = len(q_ys), len(r_ys)
    if not (0 < cnt_q <= 2 * QHALF and 0 < cnt_r <= NBLK * BLK):
        return None
    if (cnt_r + BLK - 1) // BLK != NBLK:
        return None

    s1_lhsT = _feat5_queries(np.arange(96))

    s1_rhs = np.zeros((5, NBLK, CAND, 96), np.float32)
    s1_rhs[3] = BIG  # sentinel [0, 0, 0, BIG, 0]
    s2_rtop = np.empty((5, NBLK, CAND), np.float32)
    for blk in range(NBLK):
        lo, hi = blk * BLK, min((blk + 1) * BLK, cnt_r)
        ys_b, xs_b = r_ys[lo:hi], r_xs[lo:hi]
        b0 = int(ys_b[0])
        if int(ys_b[-1]) - b0 + 1 > CAND:
            return None
        s1_rhs[:, blk, ys_b - b0, xs_b] = _feat5_refs(xs_b)
        s2_rtop[:, blk, :] = _feat5_refs(b0 + np.arange(CAND))
    s1_pack = _bf16(np.concatenate([s1_lhsT, s1_rhs.reshape(5, -1)], axis=1))

    # stage-2 lhsT rows: 0..95 onehot(x), 96..100 yfeat; padded slots zero
    s2_lhsT = np.zeros((101, 2 * QHALF), np.float32)
    s2_lhsT[q_xs, np.arange(cnt_q)] = 1.0
    s2_lhsT[96:101, :cnt_q] = _feat5_queries(q_ys)

    maps = []
    for half in range(2):
        hs = slice(half * QHALF, (half + 1) * QHALF)
        maps.append(
            {
                "s1_pack": s1_pack,
                "s2_lhsT": _bf16(s2_lhsT[:, hs]),
                "s2_rtop": _bf16(s2_rtop.reshape(5, -1)),
            }
        )
    return maps


def _quantile95(vals):
    """torch.quantile / jnp.nanquantile 'linear' on finite values."""
    v = np.sort(np.asarray(vals, np.float64))
    n = v.size
    pos = 0.95 * (n - 1)
    lo = int(np.floor(pos))
    hi = min(lo + 1, n - 1)
    frac = pos - lo
    return v[lo] * (1.0 - frac) + v[hi] * frac


def _hd95_numpy_fallback(pred, true):
    """Pure-numpy path for data outside the compiled regime."""
    p_ys, p_xs = _side_points(pred)
    t_ys, t_xs = _side_points(true)
    if len(p_ys) == 0 or len(t_ys) == 0:
        return None
    pc = np.stack([p_ys, p_xs], -1).astype(np.float32)
    tc = np.stack([t_ys, t_xs], -1).astype(np.float32)
    vals = []
    for qc, rc in ((pc, tc), (tc, pc)):
        nbr = (len(rc) + BLK - 1) // BLK
        for jb in range(nbr):
            b = rc[jb * BLK : (jb + 1) * BLK]
            d2 = (
                (qc * qc).sum(-1)[:, None]
                + (b * b).sum(-1)[None, :]
                - 2.0 * (qc @ b.T)
            )
            vals.append(np.sqrt(np.maximum(d2.min(1), 0.0).astype(np.float32)))
    return _quantile95(np.concatenate(vals))


def _run_device(in_maps, trace=False):
    from concourse.bass_utils import run_bass_kernel_spmd

    nc = _get_nc()
    return run_bass_kernel_spmd(nc, in_maps, list(range(NCORES)), trace=trace)


def kernel(input, target, _trace=False, _results_out=None):
    input = np.asarray(input)
    target = np.asarray(target)
    nimg = input.shape[0]

    # jobs: (image, direction). dir 0: queries=pred, refs=true (row mins);
    # dir 1: queries=true, refs=pred (col mins).
    jobs = []
    in_maps = []
    fallback = {}
    ok_mask = []
    for i in range(nimg):
        p_ys, p_xs = _side_points(input[i])
        t_ys, t_xs = _side_points(target[i])
        ok = len(p_ys) > 0 and len(t_ys) > 0
        ok_mask.append(ok)
        if not ok:
            continue
        built_row = _build_core_inputs(p_ys, p_xs, t_ys, t_xs)
        built_col = _build_core_inputs(t_ys, t_xs, p_ys, p_xs)
        if built_row is None or built_col is None or nimg != 2:
            fallback[i] = _hd95_numpy_fallback(input[i], target[i])
            continue
        jobs.append((i, 0, len(p_ys)))
        in_maps.extend(built_row)
        jobs.append((i, 1, len(t_ys)))
        in_maps.extend(built_col)

    hds = {}
    if jobs:
        while len(in_maps) < NCORES:  # pad to the full 8-core SPMD launch
            in_maps.append({k: v.copy() for k, v in in_maps[0].items()})
        res = _run_device(in_maps[:NCORES], trace=_trace)
        if _results_out is not None:
            _results_out.append(res)
        per_img_vals = {}
        for j, (img, _dir, cnt_q) in enumerate(jobs):
            o0 = res.results[2 * j]["mins"]      # [128, NTILES*NBLK]
            o1 = res.results[2 * j + 1]["mins"]
            d2 = np.concatenate(
                [
                    o0.reshape(128, NTILES, NBLK).transpose(1, 0, 2),
                    o1.reshape(128, NTILES, NBLK).transpose(1, 0, 2),
                ]
            ).reshape(2 * QHALF, NBLK)[:cnt_q]
            assert d2.max() < 2.0 ** 25, "sentinel leaked into mins"
            dist = np.sqrt(d2.astype(np.float32))
            per_img_vals.setdefault(img, []).append(dist.ravel())
        for img, chunks in per_img_vals.items():
            hds[img] = _quantile95(np.concatenate(chunks))
    hds.update(fallback)

    n_ok = sum(ok_mask)
    if n_ok == 0:
        return np.float32(np.inf)
    total = sum(hds[i] for i in range(nimg) if ok_mask[i])
    return np.float32(total / n_ok)



# revision 2
# speedup vs baseline: 1.5150x; 1.5150x over previous
"""HD95 loss kernel for Trainium2 (Bass/Tile), 8 NeuronCores.

Reference semantics: per image, threshold pred/true at 0.5, compact nonzero
pixel indices in row-major order, split each point list into blocks of 1000,
and for every (point, opposite-side block) pair take the min Euclidean
distance; the HD95 is the 95th linear-interpolation quantile over all finite
such mins (both directions), averaged over the batch.

Device algorithm (per image & direction, "queries" vs "ref blocks"):
separable squared-EDT with the row stage precomputed on the host.

  host:     g[x, blk, c] = min_{a : pixel(b0+c, a) in blk} (x-a)^2
            (exact integer table, bf16-rounded; sentinel 2^26 for empty
            candidate rows), plus per-candidate row features
            rtop = [1, 1, -2b, b2h, b2l] with b = b0+c (bf16-exact split
            of b^2 into a multiple of 128 plus a <128 remainder).
  device:   min d^2(q, blk) = min_c ( (y_q - (b0+c))^2 + g[x_q, c] )
            one accumulating bf16 matmul per 128-query tile:
            [onehot(x_q); y2h, y2l, y, 1, 1] @ [g ; rtop]  (K=101 rows),
            then a DVE min-reduce over the 24 candidates of each block.
            The y-part is bit-exact; g carries <=2^-9 relative rounding,
            far inside the 2e-2 harness gate.

Core mapping: 8 cores = 4 (image x direction) jobs x 2 halves of 2432
query slots. Host does the O(N) compaction/feature build and the final
O(50k) quantile; device does all O(K x window) distance work. The input
lhsT DMA is split into 16 partition-slices so it spreads across the 16
HW DMA engines (a single dma_start lands on one engine at ~15 GB/s).
"""

import numpy as np

H = 96
W = 96
BLK = 1000        # reference cdist block size
NBLK = 5          # blocks per side (asserted from the data regime)
CAND = 24         # candidate image rows per block window (spans <= 23 here)
M = NBLK * CAND   # matmul free size (120 candidate columns)
NTILES = 19       # query tiles of 128 per core
QHALF = NTILES * 128  # 2432 query slots per core
GRP = 4           # tiles per PSUM bank (4*120 f32 = 1920B <= 2KB bank)
NGRP = (NTILES + GRP - 1) // GRP  # 5 groups (last has 3 tiles)
BIG = float(2 ** 26)  # sentinel (bf16-exact, >> max real d^2 of 18050)
NCORES = 8
NDMA = 16         # input-DMA partition slices (one per HW DMA engine)

_CACHE = {}


def _build_nc():
    import concourse.bacc as bacc
    import concourse.mybir as mybir
    import concourse.tile as tile

    f32 = mybir.dt.float32
    bf16 = mybir.dt.bfloat16
    # Bacc (not raw Bass): its compile() runs move_matmul_waits_to_ldweights
    # + generate_event_semaphores, which legalize multi-wait instructions
    # (TRN2 allows at most one sync wait per instruction).
    nc = bacc.Bacc("TRN2", target_bir_lowering=False, debug=False)

    lhsT = nc.declare_dram_parameter("lhsT", [101, QHALF], bf16, isOutput=False)
    ghr = nc.declare_dram_parameter("ghr", [101, M], bf16, isOutput=False)
    mins = nc.declare_dram_parameter(
        "mins", [128, NTILES * NBLK], f32, isOutput=True
    )

    X = mybir.AxisListType.X
    MIN = mybir.AluOpType.min

    with tile.TileContext(nc) as tc:
        with (
            tc.tile_pool(name="const", bufs=1) as const,
            tc.tile_pool(name="ps", bufs=NGRP, space="PSUM") as psp,
        ):
            t_lhsT = const.tile([101, QHALF], bf16)
            t_ghr = const.tile([101, M], bf16)
            t_out = const.tile([128, NTILES * NBLK], f32)

            # rhs first (needed by every matmul), then the big lhsT split
            # across the 16 HW DMA engines (one dma_start each)
            nc.sync.dma_start(t_ghr[:], ghr[:])
            bounds = np.linspace(0, 101, NDMA + 1).astype(int)
            for i in range(NDMA):
                sl = slice(int(bounds[i]), int(bounds[i + 1]))
                nc.sync.dma_start(t_lhsT[sl, :], lhsT[sl, :])

            for g in range(NGRP):
                nt = min(GRP, NTILES - g * GRP)
                ps = psp.tile([128, GRP, NBLK, CAND], f32, tag="ps")
                for k in range(nt):
                    t = g * GRP + k
                    nc.tensor.matmul(
                        ps[:, k, :, :],
                        t_lhsT[:, t * 128 : (t + 1) * 128],
                        t_ghr[:],
                        start=True,
                        stop=True,
                    )
                o0 = g * GRP * NBLK
                nc.vector.tensor_reduce(
                    t_out[:, o0 : o0 + nt * NBLK],
                    ps[:, 0:nt, :, :],
                    axis=X,
                    op=MIN,
                )
                nc.sync.dma_start(
                    mins[:, o0 : o0 + nt * NBLK], t_out[:, o0 : o0 + nt * NBLK]
                )

    nc.compile()
    return nc


def _get_nc():
    if "nc" not in _CACHE:
        _CACHE["nc"] = _build_nc()
    return _CACHE["nc"]


def _bf16(a):
    from ml_dtypes import bfloat16

    return np.asarray(a, np.float32).astype(bfloat16)


def _hilo(v):
    """Split integer-valued array into (multiple-of-128, remainder<128)."""
    v = np.asarray(v, np.float64)
    lo = np.mod(v, 128.0)
    return (v - lo).astype(np.float32), lo.astype(np.float32)


def _side_points(img):
    """Compacted nonzero pixel coords, row-major ascending (matches
    jnp.nonzero order)."""
    m = (np.asarray(img) > 0.5).reshape(-1)
    idx = np.nonzero(m)[0]
    ys = (idx // W).astype(np.int64)
    xs = (idx % W).astype(np.int64)
    return ys, xs


def _feat5_queries(vals):
    """[v2h, v2l, v, 1, 1] feature rows for the squared-term side."""
    v = np.asarray(vals, np.float64)
    h, l = _hilo(v * v)
    one = np.ones_like(v, np.float32)
    return np.stack([h, l, v.astype(np.float32), one, one])


def _feat5_refs(vals):
    """[1, 1, -2v, v2h, v2l] feature rows for the reference side."""
    v = np.asarray(vals, np.float64)
    h, l = _hilo(v * v)
    one = np.ones_like(v, np.float32)
    return np.stack([one, one, (-2.0 * v).astype(np.float32), h, l])


def _build_core_inputs(q_ys, q_xs, r_ys, r_xs):
    """Host-side feature build for one (image, direction) job.

    q_*: query points (cnt_q), r_*: reference points (cnt_r, split into
    NBLK blocks of BLK in compacted order). Returns two per-core input
    maps, or None if the data falls outside the compiled regime.
    """
    cnt_q, cnt_r = len(q_ys), len(r_ys)
    if not (0 < cnt_q <= 2 * QHALF and 0 < cnt_r <= NBLK * BLK):
        return None
    if (cnt_r + BLK - 1) // BLK != NBLK:
        return None

    xgrid = np.arange(W, dtype=np.float64)
    g = np.full((W, NBLK, CAND), BIG, np.float32)  # rows 0..95 of rhs
    rtop = np.empty((5, NBLK, CAND), np.float32)
    for blk in range(NBLK):
        lo, hi = blk * BLK, min((blk + 1) * BLK, cnt_r)
        ys_b, xs_b = r_ys[lo:hi], r_xs[lo:hi]
        b0 = int(ys_b[0])
        if int(ys_b[-1]) - b0 + 1 > CAND:
            return None
        for c in np.unique(ys_b - b0):
            xs_c = xs_b[ys_b - b0 == c].astype(np.float64)
            d = np.abs(xgrid[:, None] - xs_c[None, :]).min(1)
            g[:, blk, c] = (d * d).astype(np.float32)
        rtop[:, blk, :] = _feat5_refs(b0 + np.arange(CAND))
    ghr = _bf16(
        np.concatenate([g.reshape(W, -1), rtop.reshape(5, -1)], axis=0)
    )

    # lhsT rows: 0..95 onehot(x), 96..100 yfeat; padded slots zero
    s2_lhsT = np.zeros((101, 2 * QHALF), np.float32)
    s2_lhsT[q_xs, np.arange(cnt_q)] = 1.0
    s2_lhsT[96:101, :cnt_q] = _feat5_queries(q_ys)

    maps = []
    for half in range(2):
        hs = slice(half * QHALF, (half + 1) * QHALF)
        maps.append({"lhsT": _bf16(s2_lhsT[:, hs]), "ghr": ghr})
    return maps


def _quantile95(vals):
    """torch.quantile / jnp.nanquantile 'linear' on finite values."""
    v = np.sort(np.asarray(vals, np.float64))
    n = v.size
    pos = 0.95 * (n - 1)
    lo = int(np.floor(pos))
    hi = min(lo + 1, n - 1)
    frac = pos - lo
    return v[lo] * (1.0 - frac) + v[hi] * frac


def _hd95_numpy_fallback(pred, true):
    """Pure-numpy path for data outside the compiled regime."""
    p_ys, p_xs = _side_points(pred)
    t_ys, t_xs = _side_points(true)
    if len(p_ys) == 0 or len(t_ys) == 0:
        return None
    pc = np.stack([p_ys, p_xs], -1).astype(np.float32)
    tc = np.stack([t_ys, t_xs], -1).astype(np.float32)
    vals = []
    for qc, rc in ((pc, tc), (tc, pc)):
        nbr = (len(rc) + BLK - 1) // BLK
        for jb in range(nbr):
            b = rc[jb * BLK : (jb + 1) * BLK]
            d2 = (
                (qc * qc).sum(-1)[:, None]
                + (b * b).sum(-1)[None, :]
                - 2.0 * (qc @ b.T)
            )
            vals.append(np.sqrt(np.maximum(d2.min(1), 0.0).astype(np.float32)))
    return _quantile95(np.concatenate(vals))


def _run_device(in_maps, trace=False):
    from concourse.bass_utils import run_bass_kernel_spmd

    nc = _get_nc()
    return run_bass_kernel_spmd(nc, in_maps, list(range(NCORES)), trace=trace)


def _decode_mins(raw):
    """[128, NTILES*NBLK] device layout -> [QHALF, NBLK] query-major d^2."""
    # column g*GRP*NBLK + k*NBLK + blk holds tile t = g*GRP+k; query
    # q = t*128 + partition
    return (
        raw.reshape(128, NTILES, NBLK).transpose(1, 0, 2).reshape(QHALF, NBLK)
    )


def kernel(input, target, _trace=False, _results_out=None):
    input = np.asarray(input)
    target = np.asarray(target)
    nimg = input.shape[0]

    # jobs: (image, direction). dir 0: queries=pred, refs=true (row mins);
    # dir 1: queries=true, refs=pred (col mins).
    jobs = []
    in_maps = []
    fallback = {}
    ok_mask = []
    for i in range(nimg):
        p_ys, p_xs = _side_points(input[i])
        t_ys, t_xs = _side_points(target[i])
        ok = len(p_ys) > 0 and len(t_ys) > 0
        ok_mask.append(ok)
        if not ok:
            continue
        built_row = _build_core_inputs(p_ys, p_xs, t_ys, t_xs)
        built_col = _build_core_inputs(t_ys, t_xs, p_ys, p_xs)
        if built_row is None or built_col is None or nimg != 2:
            fallback[i] = _hd95_numpy_fallback(input[i], target[i])
            continue
        jobs.append((i, 0, len(p_ys)))
        in_maps.extend(built_row)
        jobs.append((i, 1, len(t_ys)))
        in_maps.extend(built_col)

    hds = {}
    if jobs:
        while len(in_maps) < NCORES:  # pad to the full 8-core SPMD launch
            in_maps.append({k: v.copy() for k, v in in_maps[0].items()})
        res = _run_device(in_maps[:NCORES], trace=_trace)
        if _results_out is not None:
            _results_out.append(res)
        per_img_vals = {}
        for j, (img, _dir, cnt_q) in enumerate(jobs):
            d2 = np.concatenate(
                [
                    _decode_mins(res.results[2 * j]["mins"]),
                    _decode_mins(res.results[2 * j + 1]["mins"]),
                ]
            )[:cnt_q]
            assert d2.max() < 2.0 ** 25, "sentinel leaked into mins"
            dist = np.sqrt(d2.astype(np.float32))
            per_img_vals.setdefault(img, []).append(dist.ravel())
        for img, chunks in per_img_vals.items():
            hds[img] = _quantile95(np.concatenate(chunks))
    hds.update(fallback)

    n_ok = sum(ok_mask)
    if n_ok == 0:
        return np.float32(np.inf)
    total = sum(hds[i] for i in range(nimg) if ok_mask[i])
    return np.float32(total / n_ok)
